# revision 11
# baseline (speedup 1.0000x reference)
"""Trainium2 Bass kernel for nn_CausalAttention (B=2, S=2048, D=1024, H=16).

Sharding: tensor-parallel over heads (4 groups of 4 heads) x data-parallel
over batch (2), on 8 NeuronCores. Core c handles batch b = c // 4 and head
group g = c % 4 (heads 4g..4g+3, i.e. d_model columns 256g..256g+256).

Each core computes, fully on-device in bf16 (f32 PSUM accumulation):
  Q^T, K^T (d_out on partitions) and V (s on partitions, ones column
  appended) for its head slice, projected incrementally per q-chunk so
  projection matmuls overlap the (exp-latency-bound) attention pipeline.
  K gets NO bias: any per-q additive shift of the scores cancels in the
  softmax (numerator and denominator share the factor), and with K
  bias-free every bk-dependent term is per-q or constant.

  Scores are computed transposed (k on partitions, q free) with the two
  heads of a pair (even/odd head of a kt_sb tile) ROW-TILED on the PE
  array: head A contracts on array rows 0-63 (tile_position (0,0)), head
  B on rows 64-127 ((64,0)) -- emitted back-to-back they run
  concurrently, doubling score throughput. Causal masking of diagonal
  128-blocks is folded into the score PSUM via accumulate-matmuls
  (identity-half stationary x (-240 upper-triangle) moving, also
  row-tiled), so exp(scale*(S - 240)) == 0 -- no post-exp gpsimd pass.

  P^T = exp(S^T / 8) per (pair, kt) in one activation; unnormalized
  attention out^T[dh, q] via V-stationary matmuls where a ones column in
  V yields the softmax denominator for free; normalization by the
  broadcast fast-approx reciprocal of the denominator row; then a
  partial out = attn @ Wo_slice in bf16, interleaved per q-chunk.

Host-side glue: x is pre-transposed per batch, weights pre-sliced and
cast to bf16; the 4 bf16 partial outputs per batch are summed in f32 and
bo + bv @ Wo (the V-bias contribution, exact since softmax rows sum to
1) is added.
"""

import sys

for _p in ("/opt/trn_rl_repo",):
    if _p not in sys.path:
        sys.path.append(_p)

import ml_dtypes
import numpy as np

import concourse.bass as bass
import concourse.mybir as mybir
import concourse.tile as tile
from concourse import bacc
from concourse.bass import ds, ts
from concourse.bass_utils import run_bass_kernel_spmd

B, S, D, H, DH = 2, 2048, 1024, 16, 64
N_CORES = 8
HPC = 4  # heads per core
DSL = HPC * DH  # 256, d_model slice per core
BF16 = mybir.dt.bfloat16
F32 = mybir.dt.float32

QC = 512  # q chunk for score tiles
KT = 128  # k tile (score-tile partition dim)
NQT = S // 128  # 16 q tiles of 128
NQC = S // QC  # 4 q chunks
NKC = D // 128  # 8 contraction chunks for projections
JPQ = QC // KT  # 4 k-tiles (and q-subtiles) per q chunk
MASKVAL = -240.0  # pre-scale; *0.125 = -30 => exp == 0 in bf16/f32


def build_nc():
    nc = bacc.Bacc(
        "TRN2",
        target_bir_lowering=False,
        debug=False,
        enable_asserts=False,
        num_devices=N_CORES,
    )
    xT_ext = nc.dram_tensor("xT", [D, S], BF16, kind="ExternalInput")
    wq_ext = nc.dram_tensor("wq", [D, DSL], BF16, kind="ExternalInput")
    wk_ext = nc.dram_tensor("wk", [D, DSL], BF16, kind="ExternalInput")
    wv_ext = nc.dram_tensor("wv", [D, DSL], BF16, kind="ExternalInput")
    wo_ext = nc.dram_tensor("wo", [DSL, D], BF16, kind="ExternalInput")
    bq_ext = nc.dram_tensor("bq", [DSL], F32, kind="ExternalInput")
    eye_ext = nc.dram_tensor("eye", [128, 128], BF16, kind="ExternalInput")
    msk_ext = nc.dram_tensor("msk", [128, 128], BF16, kind="ExternalInput")
    out_ext = nc.dram_tensor("out", [S, D], BF16, kind="ExternalOutput")

    with tile.TileContext(nc) as tc:
        with (
            tc.tile_pool(name="consts", bufs=1) as consts,
            tc.tile_pool(name="weights", bufs=1) as weights,
            tc.tile_pool(name="xt", bufs=1) as xt_pool,
            tc.tile_pool(name="qkv", bufs=1) as qkv_pool,
            tc.tile_pool(name="pt", bufs=34) as pt_pool,
            tc.tile_pool(name="norm", bufs=8) as norm_pool,
            tc.tile_pool(name="out_sb", bufs=3) as out_pool,
            # PSUM budget (8 banks): 2 x 2-bank score/proj tiles + 3 x
            # 1-bank (P@V pair / out-proj) + 1 x 1-bank denominator
            tc.tile_pool(name="s_psum", bufs=2, space="PSUM") as s_psum,
            tc.tile_pool(name="o_psum", bufs=3, space="PSUM") as o_psum,
            tc.tile_pool(name="d_psum", bufs=1, space="PSUM") as d_psum,
        ):
            # --- weight / xT / const loads ---
            # The sync queue is ISSUE-limited (~0.6us per DMA), so order by
            # first use: wq, then xT chunk 0 (all the first Q-chain needs),
            # then the small consts, then the remaining xT chunks. The
            # scalar ring carries wk, wv, wo (done before exps ramp up).
            bq_sb = consts.tile([128, 2], F32, name="bq_sb")
            eye_sb = consts.tile([128, 128], BF16, name="eye_sb")
            msk_sb = consts.tile([128, 128], BF16, name="msk_sb")
            ones_sb = consts.tile([128, 1], BF16, name="ones_sb")
            wq_sb = weights.tile([128, NKC, DSL], BF16, name="wq_sb")
            wk_sb = weights.tile([128, NKC, DSL], BF16, name="wk_sb")
            wv_sb = weights.tile([128, NKC, DSL], BF16, name="wv_sb")
            wo_sb = weights.tile([128, 2, D], BF16, name="wo_sb")
            nc.sync.dma_start(
                wq_sb[:], wq_ext.ap().rearrange("(c p) n -> p c n", p=128)
            )
            nc.scalar.dma_start(
                wk_sb[:], wk_ext.ap().rearrange("(c p) n -> p c n", p=128)
            )
            nc.scalar.dma_start(
                wv_sb[:], wv_ext.ap().rearrange("(c p) n -> p c n", p=128)
            )
            nc.scalar.dma_start(
                wo_sb[:], wo_ext.ap().rearrange("(c p) n -> p c n", p=128)
            )
            xt_sb = [[None] * NQC for _ in range(NKC)]
            for sc in range(NQC):
                for kc in range(NKC):
                    xt_sb[kc][sc] = xt_pool.tile(
                        [128, QC], BF16, name=f"xt{kc}_{sc}"
                    )
            for kc in range(NKC):
                nc.sync.dma_start(
                    xt_sb[kc][0][:], xT_ext.ap()[ts(kc, 128), ts(0, QC)]
                )
            nc.sync.dma_start(bq_sb[:], bq_ext.ap().rearrange("(c p) -> p c", p=128))
            nc.sync.dma_start(eye_sb[:], eye_ext.ap())
            nc.sync.dma_start(msk_sb[:], msk_ext.ap())
            nc.gpsimd.memset(ones_sb[:], 1.0)
            for sc in range(1, NQC):
                for kc in range(NKC):
                    nc.sync.dma_start(
                        xt_sb[kc][sc][:], xT_ext.ap()[ts(kc, 128), ts(sc, QC)]
                    )

            # --- projection / attention tiles ---
            qt_sb = [qkv_pool.tile([128, S], BF16, name=f"qt{c}") for c in range(2)]
            kt_sb = [qkv_pool.tile([128, S], BF16, name=f"kt{c}") for c in range(2)]
            v_sb = [
                qkv_pool.tile([128, 2, 2 * DH], BF16, name=f"v{st}")
                for st in range(NQT)
            ]
            attnT_sb = [qkv_pool.tile([128, S], BF16, name=f"att{c}") for c in range(2)]

            def proj_chunks(sc):
                """Emit-callbacks projecting Q^T/K^T columns and V s-tiles of
                q-chunk sc, one 8-matmul chain each."""

                def qk(dst, w_sb, ci, bias):
                    def emit():
                        ps = s_psum.tile([128, 2, QC], F32, name="sc")
                        for kc in range(NKC):
                            nc.tensor.matmul(
                                ps[:, 0, :],
                                w_sb[:, kc, ts(ci, 128)],
                                xt_sb[kc][sc][:],
                                start=(kc == 0),
                                stop=(kc == NKC - 1),
                            )
                        if bias is not None:
                            nc.vector.tensor_scalar_add(
                                out=dst[ci][:, ts(sc, QC)],
                                in0=ps[:, 0, :],
                                scalar1=bias,
                            )
                        else:
                            nc.vector.tensor_copy(
                                dst[ci][:, ts(sc, QC)], ps[:, 0, :]
                            )

                    return emit

                def vproj(st):
                    def emit():
                        ps = s_psum.tile([128, 2, QC], F32, name="sc")
                        for kc in range(NKC):
                            nc.tensor.matmul(
                                ps[:, 0, 0:DSL],
                                xt_sb[kc][st // JPQ][:, ts(st % JPQ, 128)],
                                wv_sb[:, kc, :],
                                start=(kc == 0),
                                stop=(kc == NKC - 1),
                            )
                        # (gpsimd cannot read PSUM -- this stays on DVE)
                        nc.vector.tensor_copy(
                            v_sb[st][:, :, :],
                            ps[:, 0, 0:DSL].rearrange("p (g d) -> p g d", g=2),
                        )

                    return emit

                chains = [
                    qk(qt_sb, wq_sb, 0, bq_sb[:, ds(0, 1)]),
                    qk(kt_sb, wk_sb, 0, None),
                    qk(qt_sb, wq_sb, 1, bq_sb[:, ds(1, 1)]),
                    qk(kt_sb, wk_sb, 1, None),
                ]
                for st in range(JPQ * sc, JPQ * (sc + 1)):
                    chains.append(vproj(st))
                return chains

            def widths(qc):
                n_kt = (qc + 1) * JPQ
                return n_kt, [QC - KT * max(0, kt - qc * JPQ) for kt in range(n_kt)]

            def score_group(qc, ci, kt, pt_list):
                """Scores for both heads of pair ci at k-tile kt, row-tiled
                on the PE array, causal mask accumulated for diagonal
                tiles, one exp for the pair. The psum/pt tiles are
                [128, 2, 512]: head half h in slab h, so each matmul output
                stays inside one PSUM bank; the exp covers both slabs with
                a strided free AP (no gap columns)."""
                n_kt, width = widths(qc)
                w = width[kt]
                qoff = qc * QC + (QC - w)
                is_diag = kt >= qc * JPQ
                sc = s_psum.tile([128, 2, QC], F32, name="sc")
                pt = pt_pool.tile([128, 2, QC], BF16, name="pt")
                for half in range(2):
                    po = half * 64
                    nc.tensor.matmul(
                        sc[:, half, 0:w],
                        kt_sb[ci][po : po + 64, ts(kt, KT)],
                        qt_sb[ci][po : po + 64, ds(qoff, w)],
                        start=True,
                        stop=True,
                        skip_group_check=is_diag,
                    )
                if is_diag:
                    # psum[k, q-diagblock] += -240 * [k > q] as a separate
                    # single-matmul accumulation group (full-128 identity
                    # stationary; mixing tile positions inside one open
                    # accumulation group wedges the exec unit on HW)
                    for half in range(2):
                        nc.tensor.matmul(
                            sc[:, half, 0:KT],
                            eye_sb[:],
                            msk_sb[:],
                            start=False,
                            stop=True,
                            skip_group_check=True,
                        )
                nc.scalar.activation(
                    pt[:, :, 0:w],
                    sc[:, :, 0:w],
                    mybir.ActivationFunctionType.Exp,
                    scale=0.125,
                )
                pt_list[kt] = pt

            def den_span(qc, kt, den_ps, pts):
                """Softmax denominators for all 4 heads at k-tile kt: four
                col-tiled 1-column matmuls (ones stationary, P^T moving)
                accumulating into den_ps rows 0/32/64/96 across k-tiles."""
                n_kt, width = widths(qc)
                w = width[kt]
                for h in range(HPC):
                    nc.tensor.matmul(
                        den_ps[ds(32 * h, 1), ds(QC - w, w)],
                        ones_sb[:, 0:1],
                        pts[h // 2][kt][:, h % 2, 0:w],
                        start=(kt == 0),
                        stop=(kt == n_kt - 1),
                        skip_group_check=True,
                        tile_position=(0, 32 * h),
                    )

            def pv_pair(qc, ci, den_ps, pt_list):
                """P@V for both heads of pair ci, col-tiled (head A on array
                cols 0-63 -> psum partitions 0-63, head B on 64-127), then
                per-head normalization from the den_ps rows."""
                n_kt, width = widths(qc)
                po_ = o_psum.tile([128, QC], F32, name="ov")
                for kt in range(n_kt):
                    w = width[kt]
                    for half in range(2):
                        nc.tensor.matmul(
                            po_[ds(64 * half, DH), ds(QC - w, w)],
                            v_sb[kt][:, ci, ds(64 * half, DH)],
                            pt_list[kt][:, half, 0:w],
                            start=(kt == 0),
                            stop=(kt == n_kt - 1),
                            skip_group_check=True,
                        )
                for half in range(2):
                    h = ci * 2 + half
                    den = norm_pool.tile([64, QC], F32, name="den")
                    row = norm_pool.tile([1, QC], F32, name="row")
                    nc.vector.tensor_copy(row[:], den_ps[ds(32 * h, 1), :])
                    nc.vector.reciprocal_approx_fast(den[0:1, :], row[:])
                    nc.gpsimd.partition_broadcast(den[:], den[0:1, :])
                    nc.vector.tensor_mul(
                        attnT_sb[ci][half * 64 : half * 64 + 64, ts(qc, QC)],
                        po_[ds(64 * half, DH), :],
                        den[:],
                    )

            def out_proj_tile(qc, j, pool=None):
                qt = qc * JPQ + j
                o_sb = out_pool.tile([128, D], BF16, name="osb")
                for ncol in range(2):
                    if pool is None:
                        pu = o_psum.tile([128, QC], F32, name="ov")
                    else:
                        # final chunk: the scores pool is idle by now; use
                        # its 3 wide slots so all 4 tail tiles overlap
                        pu = pool.tile([128, 2, QC], F32, name="sc")[:, 0, :]
                    for ci in range(2):
                        nc.tensor.matmul(
                            pu[:],
                            attnT_sb[ci][:, ts(qt, 128)],
                            wo_sb[:, ci, ts(ncol, 512)],
                            start=(ci == 0),
                            stop=(ci == 1),
                        )
                    nc.vector.tensor_copy(o_sb[:, ts(ncol, 512)], pu[:])
                nc.sync.dma_start(out_ext.ap()[ts(qt, 128), :], o_sb[:])

            # --- schedule ---
            # Engines execute strictly in program order; the exp stream on
            # the scalar engine is ~2x slower than the packed score matmuls
            # feeding it, so independent matmul work (projection chains of
            # the next chunk, the previous chunk's out-projection, P@V of
            # already-exp'd pairs) is threaded between score groups to keep
            # the PE busy while score PSUM bufs wait on exp drains.
            c0 = proj_chunks(0)  # [Q0, K0, Q1, K1, V0..V3]
            pt_q0 = {0: [None] * JPQ, 1: [None] * JPQ}
            c0[0]()
            c0[1]()
            c0[4]()  # V0 covers the K0 psum->SBUF copy latency
            for kt in range(JPQ):
                score_group(0, 0, kt, pt_q0[0])
            c0[2]()
            c0[3]()
            c0[5]()
            for kt in range(JPQ):
                score_group(0, 1, kt, pt_q0[1])
            c0[6]()
            c0[7]()

            for qc in range(NQC):
                n_kt = (qc + 1) * JPQ
                filler = proj_chunks(qc + 1) if qc + 1 < NQC else []
                fi = 0

                def fill(n):
                    nonlocal fi
                    for _ in range(n):
                        if fi < len(filler):
                            filler[fi]()
                            fi += 1

                def oprev(j):
                    if qc > 0:
                        out_proj_tile(qc - 1, j)

                den_ps = d_psum.tile([128, QC], F32, name="dps")
                if qc == 0:
                    pts = pt_q0
                else:
                    pts = {0: [None] * n_kt, 1: [None] * n_kt}
                    # score groups with filler threaded in to pace the
                    # (exp-limited) psum recycling
                    q4 = max(1, n_kt // 4)
                    for kt in range(n_kt):
                        score_group(qc, 0, kt, pts[0])
                        if kt == q4:
                            fill(1)  # Q0
                        if kt == 3 * q4 and qc >= 2:
                            oprev(0)
                    fill(1)  # K0
                    if qc < 2:
                        oprev(0)
                    for kt in range(n_kt):
                        score_group(qc, 1, kt, pts[1])
                        if kt == q4:
                            fill(1)  # Q1
                        if kt == 3 * q4 and qc >= 2:
                            oprev(1)
                for kt in range(n_kt):
                    den_span(qc, kt, den_ps, pts)
                if qc == 0:
                    fill(2)
                elif qc < 2:
                    oprev(1)
                fill(1)  # K1 (or V for qc 0)
                pv_pair(qc, 0, den_ps, pts[0])
                fill(1)
                oprev(2)
                pv_pair(qc, 1, den_ps, pts[1])
                fill(len(filler) - fi)
                oprev(3)
            for j in range(JPQ):
                out_proj_tile(NQC - 1, j, pool=s_psum)

    nc.compile()
    return nc


_NC_CACHE = None


def _get_nc():
    global _NC_CACHE
    if _NC_CACHE is None:
        _NC_CACHE = build_nc()
    return _NC_CACHE


def make_in_maps(x, Wq, bq, Wk, bk, Wv, bv, Wo, bo):
    bf = ml_dtypes.bfloat16
    eye = np.eye(128, dtype=bf)
    msk = np.where(
        np.arange(128)[:, None] > np.arange(128)[None, :], MASKVAL, 0.0
    ).astype(bf)
    in_maps = []
    for c in range(N_CORES):
        b, g = c // HPC, c % HPC
        lo, hi = g * DSL, (g + 1) * DSL
        in_maps.append(
            {
                "xT": np.ascontiguousarray(x[b].T).astype(bf),
                "wq": np.ascontiguousarray(Wq[:, lo:hi]).astype(bf),
                "wk": np.ascontiguousarray(Wk[:, lo:hi]).astype(bf),
                "wv": np.ascontiguousarray(Wv[:, lo:hi]).astype(bf),
                "wo": np.ascontiguousarray(Wo[lo:hi, :]).astype(bf),
                "bq": np.ascontiguousarray(bq[lo:hi]).astype(np.float32),
                "eye": eye,
                "msk": msk,
            }
        )
    return in_maps


def gather_output(results, bv, Wo, bo):
    # softmax rows sum to 1, so the V-bias contributes bv @ Wo to every row
    corr = (np.asarray(bv, np.float64) @ np.asarray(Wo, np.float64)).astype(
        np.float32
    ) + np.asarray(bo, np.float32)
    out = np.empty((B, S, D), np.float32)
    for b in range(B):
        acc = np.zeros((S, D), np.float32)
        for g in range(HPC):
            acc += results[b * HPC + g]["out"].astype(np.float32)
        out[b] = acc + corr
    return out


def kernel(x, Wq, bq, Wk, bk, Wv, bv, Wo, bo, _trace=False):
    x = np.asarray(x, np.float32)
    nc = _get_nc()
    in_maps = make_in_maps(x, Wq, bq, Wk, bk, Wv, bv, Wo, bo)
    res = run_bass_kernel_spmd(nc, in_maps, list(range(N_CORES)), trace=_trace)
    out = gather_output(res.results, bv, Wo, bo)
    if _trace:
        return out, res
    return out


# revision 13
# speedup vs baseline: 1.1321x; 1.1321x over previous
"""Trainium2 Bass kernel for nn_CausalAttention (B=2, S=2048, D=1024, H=16).

Sharding: tensor-parallel over heads (4 groups of 4 heads) x data-parallel
over batch (2), on 8 NeuronCores. Core c handles batch b = c // 4 and head
group g = c % 4 (heads 4g..4g+3, i.e. d_model columns 256g..256g+256).

Each core computes, fully on-device in bf16 (f32 PSUM accumulation):
  Q^T, K^T (d_out on partitions) and V (s on partitions, ones column
  appended) for its head slice, projected incrementally per q-chunk so
  projection matmuls overlap the (exp-latency-bound) attention pipeline.
  K gets NO bias: any per-q additive shift of the scores cancels in the
  softmax (numerator and denominator share the factor), and with K
  bias-free every bk-dependent term is per-q or constant.

  Scores are computed transposed (k on partitions, q free) with the two
  heads of a pair (even/odd head of a kt_sb tile) ROW-TILED on the PE
  array: head A contracts on array rows 0-63 (tile_position (0,0)), head
  B on rows 64-127 ((64,0)) -- emitted back-to-back they run
  concurrently, doubling score throughput. Causal masking of diagonal
  128-blocks is folded into the score PSUM via accumulate-matmuls
  (identity-half stationary x (-240 upper-triangle) moving, also
  row-tiled), so exp(scale*(S - 240)) == 0 -- no post-exp gpsimd pass.

  P^T = exp(S^T / 8) per (pair, kt) in one activation; unnormalized
  attention out^T[dh, q] via V-stationary matmuls where a ones column in
  V yields the softmax denominator for free; normalization by the
  broadcast fast-approx reciprocal of the denominator row; then a
  partial out = attn @ Wo_slice in bf16, interleaved per q-chunk.

Host-side glue: x is pre-transposed per batch, weights pre-sliced and
cast to bf16; the 4 bf16 partial outputs per batch are summed in f32 and
bo + bv @ Wo (the V-bias contribution, exact since softmax rows sum to
1) is added.
"""

import sys

for _p in ("/opt/trn_rl_repo",):
    if _p not in sys.path:
        sys.path.append(_p)

import ml_dtypes
import numpy as np

import concourse.bass as bass
import concourse.mybir as mybir
import concourse.tile as tile
from concourse import bacc
from concourse.bass import ds, ts
from concourse.bass_utils import run_bass_kernel_spmd

B, S, D, H, DH = 2, 2048, 1024, 16, 64
N_CORES = 8
HPC = 4  # heads per core
DSL = HPC * DH  # 256, d_model slice per core
BF16 = mybir.dt.bfloat16
F32 = mybir.dt.float32

QC = 512  # q chunk for score tiles
KT = 128  # k tile (score-tile partition dim)
NQT = S // 128  # 16 q tiles of 128
NQC = S // QC  # 4 q chunks
NKC = D // 128  # 8 contraction chunks for projections
JPQ = QC // KT  # 4 k-tiles (and q-subtiles) per q chunk
MASKVAL = -240.0  # pre-scale; *0.125 = -30 => exp == 0 in bf16/f32


def build_nc():
    nc = bacc.Bacc(
        "TRN2",
        target_bir_lowering=False,
        debug=False,
        enable_asserts=False,
        num_devices=N_CORES,
    )
    xT_ext = nc.dram_tensor("xT", [D, S], BF16, kind="ExternalInput")
    wq_ext = nc.dram_tensor("wq", [D, DSL], BF16, kind="ExternalInput")
    wk_ext = nc.dram_tensor("wk", [D, DSL], BF16, kind="ExternalInput")
    wv_ext = nc.dram_tensor("wv", [D, DSL], BF16, kind="ExternalInput")
    wo_ext = nc.dram_tensor("wo", [DSL, D], BF16, kind="ExternalInput")
    bq_ext = nc.dram_tensor("bq", [DSL], F32, kind="ExternalInput")
    eye_ext = nc.dram_tensor("eye", [128, 128], BF16, kind="ExternalInput")
    msk_ext = nc.dram_tensor("msk", [128, 128], BF16, kind="ExternalInput")
    out_ext = nc.dram_tensor("out", [S, D], BF16, kind="ExternalOutput")

    with tile.TileContext(nc) as tc:
        with (
            tc.tile_pool(name="consts", bufs=1) as consts,
            tc.tile_pool(name="weights", bufs=1) as weights,
            tc.tile_pool(name="xt", bufs=1) as xt_pool,
            tc.tile_pool(name="qkv", bufs=1) as qkv_pool,
            tc.tile_pool(name="pt", bufs=40) as pt_pool,
            tc.tile_pool(name="norm", bufs=8) as norm_pool,
            tc.tile_pool(name="out_sb", bufs=3) as out_pool,
            # PSUM: 3 x 2-bank (scores/proj) + 2 x 1-bank (P@V / out-proj)
            tc.tile_pool(name="s_psum", bufs=3, space="PSUM") as s_psum,
            tc.tile_pool(name="o_psum", bufs=2, space="PSUM") as o_psum,
        ):
            # --- weight / xT / const loads ---
            # The sync queue is ISSUE-limited (~0.6us per DMA), so order by
            # first use: wq, then xT chunk 0 (all the first Q-chain needs),
            # then the small consts, then the remaining xT chunks. The
            # scalar ring carries wk, wv, wo (done before exps ramp up).
            bq_sb = consts.tile([128, 2], F32, name="bq_sb")
            eye_sb = consts.tile([128, 128], BF16, name="eye_sb")
            msk_sb = consts.tile([128, 128], BF16, name="msk_sb")
            wq_sb = weights.tile([128, NKC, DSL], BF16, name="wq_sb")
            wk_sb = weights.tile([128, NKC, DSL], BF16, name="wk_sb")
            wv_sb = weights.tile([128, NKC, DSL], BF16, name="wv_sb")
            wo_sb = weights.tile([128, 2, D], BF16, name="wo_sb")
            nc.sync.dma_start(
                wq_sb[:], wq_ext.ap().rearrange("(c p) n -> p c n", p=128)
            )
            nc.scalar.dma_start(
                wk_sb[:], wk_ext.ap().rearrange("(c p) n -> p c n", p=128)
            )
            nc.scalar.dma_start(
                wv_sb[:], wv_ext.ap().rearrange("(c p) n -> p c n", p=128)
            )
            nc.scalar.dma_start(
                wo_sb[:], wo_ext.ap().rearrange("(c p) n -> p c n", p=128)
            )
            xt_sb = [[None] * NQC for _ in range(NKC)]
            for sc in range(NQC):
                for kc in range(NKC):
                    xt_sb[kc][sc] = xt_pool.tile(
                        [128, QC], BF16, name=f"xt{kc}_{sc}"
                    )
            for kc in range(NKC):
                nc.sync.dma_start(
                    xt_sb[kc][0][:], xT_ext.ap()[ts(kc, 128), ts(0, QC)]
                )
            nc.sync.dma_start(bq_sb[:], bq_ext.ap().rearrange("(c p) -> p c", p=128))
            nc.sync.dma_start(eye_sb[:], eye_ext.ap())
            nc.sync.dma_start(msk_sb[:], msk_ext.ap())
            for sc in range(1, NQC):
                for kc in range(NKC):
                    nc.sync.dma_start(
                        xt_sb[kc][sc][:], xT_ext.ap()[ts(kc, 128), ts(sc, QC)]
                    )

            # --- projection / attention tiles ---
            qt_sb = [qkv_pool.tile([128, S], BF16, name=f"qt{c}") for c in range(2)]
            kt_sb = [qkv_pool.tile([128, S], BF16, name=f"kt{c}") for c in range(2)]
            v_sb = [
                qkv_pool.tile([128, HPC, DH + 1], BF16, name=f"v{st}")
                for st in range(NQT)
            ]
            attnT_sb = [qkv_pool.tile([128, S], BF16, name=f"att{c}") for c in range(2)]

            def proj_chunks(sc):
                """Emit-callbacks projecting Q^T/K^T columns and V s-tiles of
                q-chunk sc, one 8-matmul chain each."""

                def qk(dst, w_sb, ci, bias):
                    def emit():
                        ps = s_psum.tile([128, 2, QC], F32, name="sc")
                        for kc in range(NKC):
                            nc.tensor.matmul(
                                ps[:, 0, :],
                                w_sb[:, kc, ts(ci, 128)],
                                xt_sb[kc][sc][:],
                                start=(kc == 0),
                                stop=(kc == NKC - 1),
                            )
                        if bias is not None:
                            nc.vector.tensor_scalar_add(
                                out=dst[ci][:, ts(sc, QC)],
                                in0=ps[:, 0, :],
                                scalar1=bias,
                            )
                        else:
                            nc.vector.tensor_copy(
                                dst[ci][:, ts(sc, QC)], ps[:, 0, :]
                            )

                    return emit

                def vproj(st):
                    def emit():
                        ps = s_psum.tile([128, 2, QC], F32, name="sc")
                        for kc in range(NKC):
                            nc.tensor.matmul(
                                ps[:, 0, 0:DSL],
                                xt_sb[kc][st // JPQ][:, ts(st % JPQ, 128)],
                                wv_sb[:, kc, :],
                                start=(kc == 0),
                                stop=(kc == NKC - 1),
                            )
                        # (gpsimd cannot read PSUM -- this stays on DVE)
                        nc.vector.tensor_copy(
                            v_sb[st][:, :, 0:DH],
                            ps[:, 0, 0:DSL].rearrange("p (h d) -> p h d", h=HPC),
                        )
                        nc.gpsimd.memset(v_sb[st][:, :, DH : DH + 1], 1.0)

                    return emit

                chains = [
                    qk(qt_sb, wq_sb, 0, bq_sb[:, ds(0, 1)]),
                    qk(kt_sb, wk_sb, 0, None),
                    qk(qt_sb, wq_sb, 1, bq_sb[:, ds(1, 1)]),
                    qk(kt_sb, wk_sb, 1, None),
                ]
                for st in range(JPQ * sc, JPQ * (sc + 1)):
                    chains.append(vproj(st))
                return chains

            def widths(qc):
                n_kt = (qc + 1) * JPQ
                return n_kt, [QC - KT * max(0, kt - qc * JPQ) for kt in range(n_kt)]

            def score_group(qc, ci, kt, pt_list):
                """Scores for both heads of pair ci at k-tile kt, row-tiled
                on the PE array, causal mask accumulated for diagonal
                tiles, one exp for the pair. The psum/pt tiles are
                [128, 2, 512]: head half h in slab h, so each matmul output
                stays inside one PSUM bank; the exp covers both slabs with
                a strided free AP (no gap columns)."""
                n_kt, width = widths(qc)
                w = width[kt]
                qoff = qc * QC + (QC - w)
                is_diag = kt >= qc * JPQ
                sc = s_psum.tile([128, 2, QC], F32, name="sc")
                pt = pt_pool.tile([128, 2, QC], BF16, name="pt")
                for half in range(2):
                    po = half * 64
                    nc.tensor.matmul(
                        sc[:, half, 0:w],
                        kt_sb[ci][po : po + 64, ts(kt, KT)],
                        qt_sb[ci][po : po + 64, ds(qoff, w)],
                        start=True,
                        stop=True,
                        skip_group_check=is_diag,
                    )
                if is_diag:
                    # psum[k, q-diagblock] += -240 * [k > q] as a separate
                    # single-matmul accumulation group (full-128 identity
                    # stationary; mixing tile positions inside one open
                    # accumulation group wedges the exec unit on HW)
                    for half in range(2):
                        nc.tensor.matmul(
                            sc[:, half, 0:KT],
                            eye_sb[:],
                            msk_sb[:],
                            start=False,
                            stop=True,
                            skip_group_check=True,
                        )
                nc.scalar.activation(
                    pt[:, :, 0:w],
                    sc[:, :, 0:w],
                    mybir.ActivationFunctionType.Exp,
                    scale=0.125,
                )
                pt_list[kt] = pt

            def pv_chain(qc, ci, half, pt_list, po_, kts):
                n_kt, width = widths(qc)
                h = ci * 2 + half
                for kt in kts:
                    w = width[kt]
                    nc.tensor.matmul(
                        po_[0 : DH + 1, ds(QC - w, w)],
                        v_sb[kt][:, h, :],
                        pt_list[kt][:, half, 0:w],
                        start=(kt == 0),
                        stop=(kt == n_kt - 1),
                    )

            def pv_norm(qc, ci, half, pt_list):
                po = half * 64
                n_kt, _ = widths(qc)
                po_ = o_psum.tile([128, QC], F32, name="ov")
                pv_chain(qc, ci, half, pt_list, po_, range(n_kt))
                # normalize: attnT[h rows, qc cols] = out^T * (1/denom)
                den = norm_pool.tile([64, QC], F32, name="den")
                row = norm_pool.tile([1, QC], F32, name="row")
                nc.vector.tensor_copy(row[:], po_[DH : DH + 1, :])
                nc.vector.reciprocal_approx_fast(den[0:1, :], row[:])
                nc.gpsimd.partition_broadcast(den[:], den[0:1, :])
                nc.vector.tensor_mul(
                    attnT_sb[ci][po : po + 64, ts(qc, QC)],
                    po_[0:DH, :],
                    den[:],
                )

            def out_proj_tile(qc, j, pool=None):
                qt = qc * JPQ + j
                o_sb = out_pool.tile([128, D], BF16, name="osb")
                for ncol in range(2):
                    if pool is None:
                        pu = o_psum.tile([128, QC], F32, name="ov")
                    else:
                        # final chunk: the scores pool is idle by now; use
                        # its 3 wide slots so all 4 tail tiles overlap
                        pu = pool.tile([128, 2, QC], F32, name="sc")[:, 0, :]
                    for ci in range(2):
                        nc.tensor.matmul(
                            pu[:],
                            attnT_sb[ci][:, ts(qt, 128)],
                            wo_sb[:, ci, ts(ncol, 512)],
                            start=(ci == 0),
                            stop=(ci == 1),
                        )
                    nc.vector.tensor_copy(o_sb[:, ts(ncol, 512)], pu[:])
                nc.sync.dma_start(out_ext.ap()[ts(qt, 128), :], o_sb[:])

            # --- schedule ---
            # Engines execute strictly in program order; the exp stream on
            # the scalar engine is ~2x slower than the packed score matmuls
            # feeding it, so independent matmul work (projection chains of
            # the next chunk, the previous chunk's out-projection, P@V of
            # already-exp'd pairs) is threaded between score groups to keep
            # the PE busy while score PSUM bufs wait on exp drains.
            c0 = proj_chunks(0)  # [Q0, K0, Q1, K1, V0..V3]
            pt_q0 = {0: [None] * JPQ, 1: [None] * JPQ}
            c0[0]()
            c0[1]()
            c0[4]()  # V0 covers the K0 psum->SBUF copy latency
            for kt in range(JPQ):
                score_group(0, 0, kt, pt_q0[0])
            c0[2]()
            c0[3]()
            c0[5]()
            for kt in range(JPQ):
                score_group(0, 1, kt, pt_q0[1])
            c0[6]()
            c0[7]()

            # Cross-chunk software pipeline: chunk qc+1's pair-0 score
            # groups (and their exps) are emitted during chunk qc, using the
            # scalar-engine slack of the PE-bound early chunks to pre-drain
            # the exp hump of the (otherwise exp-bound) later chunks.
            pts_cur = pt_q0
            for qc in range(NQC):
                n_kt = (qc + 1) * JPQ
                filler = proj_chunks(qc + 1) if qc + 1 < NQC else []
                fi = 0
                nxt = (
                    {0: [None] * (qc + 2) * JPQ, 1: [None] * (qc + 2) * JPQ}
                    if qc + 1 < NQC
                    else None
                )
                sgn_i = 0

                def fill(n):
                    nonlocal fi
                    for _ in range(n):
                        if fi < len(filler):
                            filler[fi]()
                            fi += 1

                def sgn(n):
                    nonlocal sgn_i
                    if nxt is None:
                        return
                    for _ in range(n):
                        if sgn_i < (qc + 2) * JPQ:
                            score_group(qc + 1, 0, sgn_i, nxt[0])
                            sgn_i += 1

                def oprev(j):
                    if qc > 0:
                        out_proj_tile(qc - 1, j)

                if qc > 0:
                    q4 = max(1, n_kt // 4)
                    for kt in range(n_kt):
                        score_group(qc, 1, kt, pts_cur[1])
                        if kt == q4:
                            fill(1)  # Q0 of qc+1
                        if kt == 2 * q4:
                            oprev(0)
                        if kt == 3 * q4:
                            fill(1)  # K0 of qc+1
                else:
                    fill(2)  # Q0, K0 of chunk 1
                pv_norm(qc, 0, 0, pts_cur[0])
                sgn(3)
                fill(1)  # Q1
                oprev(1)
                pv_norm(qc, 0, 1, pts_cur[0])
                sgn(3)
                fill(1)  # K1
                oprev(2)
                pv_norm(qc, 1, 0, pts_cur[1])
                sgn(3)
                fill(2)  # V0, V1
                pv_norm(qc, 1, 1, pts_cur[1])
                sgn(3)
                fill(len(filler) - fi)
                sgn(99)
                oprev(3)
                if nxt is not None:
                    pts_cur = nxt
            for j in range(JPQ):
                out_proj_tile(NQC - 1, j, pool=s_psum)

    nc.compile()
    return nc


_NC_CACHE = None


def _get_nc():
    global _NC_CACHE
    if _NC_CACHE is None:
        _NC_CACHE = build_nc()
    return _NC_CACHE


def make_in_maps(x, Wq, bq, Wk, bk, Wv, bv, Wo, bo):
    bf = ml_dtypes.bfloat16
    eye = np.eye(128, dtype=bf)
    msk = np.where(
        np.arange(128)[:, None] > np.arange(128)[None, :], MASKVAL, 0.0
    ).astype(bf)
    in_maps = []
    for c in range(N_CORES):
        b, g = c // HPC, c % HPC
        lo, hi = g * DSL, (g + 1) * DSL
        in_maps.append(
            {
                "xT": np.ascontiguousarray(x[b].T).astype(bf),
                "wq": np.ascontiguousarray(Wq[:, lo:hi]).astype(bf),
                "wk": np.ascontiguousarray(Wk[:, lo:hi]).astype(bf),
                "wv": np.ascontiguousarray(Wv[:, lo:hi]).astype(bf),
                "wo": np.ascontiguousarray(Wo[lo:hi, :]).astype(bf),
                "bq": np.ascontiguousarray(bq[lo:hi]).astype(np.float32),
                "eye": eye,
                "msk": msk,
            }
        )
    return in_maps


def gather_output(results, bv, Wo, bo):
    # softmax rows sum to 1, so the V-bias contributes bv @ Wo to every row
    corr = (np.asarray(bv, np.float64) @ np.asarray(Wo, np.float64)).astype(
        np.float32
    ) + np.asarray(bo, np.float32)
    out = np.empty((B, S, D), np.float32)
    for b in range(B):
        acc = np.zeros((S, D), np.float32)
        for g in range(HPC):
            acc += results[b * HPC + g]["out"].astype(np.float32)
        out[b] = acc + corr
    return out


def kernel(x, Wq, bq, Wk, bk, Wv, bv, Wo, bo, _trace=False):
    x = np.asarray(x, np.float32)
    nc = _get_nc()
    in_maps = make_in_maps(x, Wq, bq, Wk, bk, Wv, bv, Wo, bo)
    res = run_bass_kernel_spmd(nc, in_maps, list(range(N_CORES)), trace=_trace)
    out = gather_output(res.results, bv, Wo, bo)
    if _trace:
        return out, res
    return out


# revision 14
# speedup vs baseline: 1.1507x; 1.0165x over previous
"""Trainium2 Bass kernel for nn_CausalAttention (B=2, S=2048, D=1024, H=16).

Sharding: tensor-parallel over heads (4 groups of 4 heads) x data-parallel
over batch (2), on 8 NeuronCores. Core c handles batch b = c // 4 and head
group g = c % 4 (heads 4g..4g+3, i.e. d_model columns 256g..256g+256).

Each core computes, fully on-device in bf16 (f32 PSUM accumulation):
  Q^T, K^T (d_out on partitions) and V (s on partitions, ones column
  appended) for its head slice, projected incrementally per q-chunk so
  projection matmuls overlap the (exp-latency-bound) attention pipeline.
  K gets NO bias: any per-q additive shift of the scores cancels in the
  softmax (numerator and denominator share the factor), and with K
  bias-free every bk-dependent term is per-q or constant.

  Scores are computed transposed (k on partitions, q free) with the two
  heads of a pair (even/odd head of a kt_sb tile) ROW-TILED on the PE
  array: head A contracts on array rows 0-63 (tile_position (0,0)), head
  B on rows 64-127 ((64,0)) -- emitted back-to-back they run
  concurrently, doubling score throughput. Causal masking of diagonal
  128-blocks is folded into the score PSUM via accumulate-matmuls
  (identity-half stationary x (-240 upper-triangle) moving, also
  row-tiled), so exp(scale*(S - 240)) == 0 -- no post-exp gpsimd pass.

  P^T = exp(S^T / 8) per (pair, kt) in one activation; unnormalized
  attention out^T[dh, q] via V-stationary matmuls where a ones column in
  V yields the softmax denominator for free; normalization by the
  broadcast fast-approx reciprocal of the denominator row; then a
  partial out = attn @ Wo_slice in bf16, interleaved per q-chunk.

Host-side glue: x is pre-transposed per batch, weights pre-sliced and
cast to bf16; the 4 bf16 partial outputs per batch are summed in f32 and
bo + bv @ Wo (the V-bias contribution, exact since softmax rows sum to
1) is added.
"""

import sys

for _p in ("/opt/trn_rl_repo",):
    if _p not in sys.path:
        sys.path.append(_p)

import ml_dtypes
import numpy as np

import concourse.bass as bass
import concourse.mybir as mybir
import concourse.tile as tile
from concourse import bacc
from concourse.bass import ds, ts
from concourse.bass_utils import run_bass_kernel_spmd

B, S, D, H, DH = 2, 2048, 1024, 16, 64
N_CORES = 8
HPC = 4  # heads per core
DSL = HPC * DH  # 256, d_model slice per core
BF16 = mybir.dt.bfloat16
F32 = mybir.dt.float32

QC = 512  # q chunk for score tiles
KT = 128  # k tile (score-tile partition dim)
NQT = S // 128  # 16 q tiles of 128
NQC = S // QC  # 4 q chunks
NKC = D // 128  # 8 contraction chunks for projections
JPQ = QC // KT  # 4 k-tiles (and q-subtiles) per q chunk
MASKVAL = -240.0  # pre-scale; *0.125 = -30 => exp == 0 in bf16/f32


def build_nc():
    nc = bacc.Bacc(
        "TRN2",
        target_bir_lowering=False,
        debug=False,
        enable_asserts=False,
        num_devices=N_CORES,
    )
    xT_ext = nc.dram_tensor("xT", [D, S], BF16, kind="ExternalInput")
    wq_ext = nc.dram_tensor("wq", [D, DSL], BF16, kind="ExternalInput")
    wk_ext = nc.dram_tensor("wk", [D, DSL], BF16, kind="ExternalInput")
    wv_ext = nc.dram_tensor("wv", [D, DSL], BF16, kind="ExternalInput")
    wo_ext = nc.dram_tensor("wo", [DSL, D], BF16, kind="ExternalInput")
    bq_ext = nc.dram_tensor("bq", [DSL], F32, kind="ExternalInput")
    eye_ext = nc.dram_tensor("eye", [128, 128], BF16, kind="ExternalInput")
    msk_ext = nc.dram_tensor("msk", [128, 128], BF16, kind="ExternalInput")
    out_ext = nc.dram_tensor("out", [S, D], BF16, kind="ExternalOutput")

    with tile.TileContext(nc) as tc:
        with (
            tc.tile_pool(name="consts", bufs=1) as consts,
            tc.tile_pool(name="weights", bufs=1) as weights,
            tc.tile_pool(name="xt", bufs=1) as xt_pool,
            tc.tile_pool(name="qkv", bufs=1) as qkv_pool,
            tc.tile_pool(name="pt", bufs=40) as pt_pool,
            tc.tile_pool(name="norm", bufs=8) as norm_pool,
            tc.tile_pool(name="out_sb", bufs=3) as out_pool,
            # PSUM: 3 x 2-bank (scores/proj) + 2 x 1-bank (P@V / out-proj)
            tc.tile_pool(name="s_psum", bufs=3, space="PSUM") as s_psum,
            tc.tile_pool(name="o_psum", bufs=2, space="PSUM") as o_psum,
        ):
            # --- weight / xT / const loads ---
            # The sync queue is ISSUE-limited (~0.6us per DMA), so order by
            # first use: wq, then xT chunk 0 (all the first Q-chain needs),
            # then the small consts, then the remaining xT chunks. The
            # scalar ring carries wk, wv, wo (done before exps ramp up).
            bq_sb = consts.tile([128, 2], F32, name="bq_sb")
            eye_sb = consts.tile([128, 128], BF16, name="eye_sb")
            msk_sb = consts.tile([128, 128], BF16, name="msk_sb")
            wq_sb = weights.tile([128, NKC, DSL], BF16, name="wq_sb")
            wk_sb = weights.tile([128, NKC, DSL], BF16, name="wk_sb")
            wv_sb = weights.tile([128, NKC, DSL], BF16, name="wv_sb")
            wo_sb = weights.tile([128, 2, D], BF16, name="wo_sb")
            # wq head (first 2 contraction chunks) goes first so the Q
            # chain starts ASAP; wo is deferred to the sync tail (first
            # needed ~40us in); only wk+wv contend on the scalar ring.
            wq_dram = wq_ext.ap().rearrange("(c p) n -> p c n", p=128)
            nc.sync.dma_start(wq_sb[:, 0:2, :], wq_dram[:, 0:2, :])
            nc.scalar.dma_start(
                wk_sb[:], wk_ext.ap().rearrange("(c p) n -> p c n", p=128)
            )
            nc.scalar.dma_start(
                wv_sb[:], wv_ext.ap().rearrange("(c p) n -> p c n", p=128)
            )
            xt_sb = [[None] * NQC for _ in range(NKC)]
            for sc in range(NQC):
                for kc in range(NKC):
                    xt_sb[kc][sc] = xt_pool.tile(
                        [128, QC], BF16, name=f"xt{kc}_{sc}"
                    )
            for kc in range(2):
                nc.sync.dma_start(
                    xt_sb[kc][0][:], xT_ext.ap()[ts(kc, 128), ts(0, QC)]
                )
            nc.sync.dma_start(wq_sb[:, 2:NKC, :], wq_dram[:, 2:NKC, :])
            for kc in range(2, NKC):
                nc.sync.dma_start(
                    xt_sb[kc][0][:], xT_ext.ap()[ts(kc, 128), ts(0, QC)]
                )
            nc.sync.dma_start(bq_sb[:], bq_ext.ap().rearrange("(c p) -> p c", p=128))
            nc.sync.dma_start(eye_sb[:], eye_ext.ap())
            nc.sync.dma_start(msk_sb[:], msk_ext.ap())
            nc.sync.dma_start(
                wo_sb[:], wo_ext.ap().rearrange("(c p) n -> p c n", p=128)
            )
            for sc in range(1, NQC):
                for kc in range(NKC):
                    nc.sync.dma_start(
                        xt_sb[kc][sc][:], xT_ext.ap()[ts(kc, 128), ts(sc, QC)]
                    )

            # --- projection / attention tiles ---
            qt_sb = [qkv_pool.tile([128, S], BF16, name=f"qt{c}") for c in range(2)]
            kt_sb = [qkv_pool.tile([128, S], BF16, name=f"kt{c}") for c in range(2)]
            v_sb = [
                qkv_pool.tile([128, HPC, DH + 1], BF16, name=f"v{st}")
                for st in range(NQT)
            ]
            attnT_sb = [qkv_pool.tile([128, S], BF16, name=f"att{c}") for c in range(2)]

            def proj_chunks(sc):
                """Emit-callbacks projecting Q^T/K^T columns and V s-tiles of
                q-chunk sc, one 8-matmul chain each."""

                def qk(dst, w_sb, ci, bias):
                    def emit():
                        ps = s_psum.tile([128, 2, QC], F32, name="sc")
                        for kc in range(NKC):
                            nc.tensor.matmul(
                                ps[:, 0, :],
                                w_sb[:, kc, ts(ci, 128)],
                                xt_sb[kc][sc][:],
                                start=(kc == 0),
                                stop=(kc == NKC - 1),
                            )
                        if bias is not None:
                            nc.vector.tensor_scalar_add(
                                out=dst[ci][:, ts(sc, QC)],
                                in0=ps[:, 0, :],
                                scalar1=bias,
                            )
                        else:
                            nc.vector.tensor_copy(
                                dst[ci][:, ts(sc, QC)], ps[:, 0, :]
                            )

                    return emit

                def vproj(st):
                    def emit():
                        ps = s_psum.tile([128, 2, QC], F32, name="sc")
                        for kc in range(NKC):
                            nc.tensor.matmul(
                                ps[:, 0, 0:DSL],
                                xt_sb[kc][st // JPQ][:, ts(st % JPQ, 128)],
                                wv_sb[:, kc, :],
                                start=(kc == 0),
                                stop=(kc == NKC - 1),
                            )
                        # (gpsimd cannot read PSUM -- this stays on DVE)
                        nc.vector.tensor_copy(
                            v_sb[st][:, :, 0:DH],
                            ps[:, 0, 0:DSL].rearrange("p (h d) -> p h d", h=HPC),
                        )
                        nc.gpsimd.memset(v_sb[st][:, :, DH : DH + 1], 1.0)

                    return emit

                chains = [
                    qk(qt_sb, wq_sb, 0, bq_sb[:, ds(0, 1)]),
                    qk(kt_sb, wk_sb, 0, None),
                    qk(qt_sb, wq_sb, 1, bq_sb[:, ds(1, 1)]),
                    qk(kt_sb, wk_sb, 1, None),
                ]
                for st in range(JPQ * sc, JPQ * (sc + 1)):
                    chains.append(vproj(st))
                return chains

            def widths(qc):
                n_kt = (qc + 1) * JPQ
                return n_kt, [QC - KT * max(0, kt - qc * JPQ) for kt in range(n_kt)]

            def score_group(qc, ci, kt, pt_list):
                """Scores for both heads of pair ci at k-tile kt, row-tiled
                on the PE array, causal mask accumulated for diagonal
                tiles, one exp for the pair. The psum/pt tiles are
                [128, 2, 512]: head half h in slab h, so each matmul output
                stays inside one PSUM bank; the exp covers both slabs with
                a strided free AP (no gap columns)."""
                n_kt, width = widths(qc)
                w = width[kt]
                qoff = qc * QC + (QC - w)
                is_diag = kt >= qc * JPQ
                sc = s_psum.tile([128, 2, QC], F32, name="sc")
                pt = pt_pool.tile([128, 2, QC], BF16, name="pt")
                for half in range(2):
                    po = half * 64
                    nc.tensor.matmul(
                        sc[:, half, 0:w],
                        kt_sb[ci][po : po + 64, ts(kt, KT)],
                        qt_sb[ci][po : po + 64, ds(qoff, w)],
                        start=True,
                        stop=True,
                        skip_group_check=is_diag,
                    )
                if is_diag:
                    # psum[k, q-diagblock] += -240 * [k > q] as a separate
                    # single-matmul accumulation group (full-128 identity
                    # stationary; mixing tile positions inside one open
                    # accumulation group wedges the exec unit on HW)
                    for half in range(2):
                        nc.tensor.matmul(
                            sc[:, half, 0:KT],
                            eye_sb[:],
                            msk_sb[:],
                            start=False,
                            stop=True,
                            skip_group_check=True,
                        )
                nc.scalar.activation(
                    pt[:, :, 0:w],
                    sc[:, :, 0:w],
                    mybir.ActivationFunctionType.Exp,
                    scale=0.125,
                )
                pt_list[kt] = pt

            def pv_chain(qc, ci, half, pt_list, po_, kts):
                n_kt, width = widths(qc)
                h = ci * 2 + half
                for kt in kts:
                    w = width[kt]
                    nc.tensor.matmul(
                        po_[0 : DH + 1, ds(QC - w, w)],
                        v_sb[kt][:, h, :],
                        pt_list[kt][:, half, 0:w],
                        start=(kt == 0),
                        stop=(kt == n_kt - 1),
                    )

            def pv_norm(qc, ci, half, pt_list):
                po = half * 64
                n_kt, _ = widths(qc)
                po_ = o_psum.tile([128, QC], F32, name="ov")
                pv_chain(qc, ci, half, pt_list, po_, range(n_kt))
                # normalize: attnT[h rows, qc cols] = out^T * (1/denom)
                den = norm_pool.tile([64, QC], F32, name="den")
                row = norm_pool.tile([1, QC], F32, name="row")
                nc.vector.tensor_copy(row[:], po_[DH : DH + 1, :])
                nc.vector.reciprocal_approx_fast(den[0:1, :], row[:])
                nc.gpsimd.partition_broadcast(den[:], den[0:1, :])
                nc.vector.tensor_mul(
                    attnT_sb[ci][po : po + 64, ts(qc, QC)],
                    po_[0:DH, :],
                    den[:],
                )

            def out_proj_tile(qc, j, pool=None):
                qt = qc * JPQ + j
                o_sb = out_pool.tile([128, D], BF16, name="osb")
                for ncol in range(2):
                    if pool is None:
                        pu = o_psum.tile([128, QC], F32, name="ov")
                    else:
                        # final chunk: the scores pool is idle by now; use
                        # its 3 wide slots so all 4 tail tiles overlap
                        pu = pool.tile([128, 2, QC], F32, name="sc")[:, 0, :]
                    for ci in range(2):
                        nc.tensor.matmul(
                            pu[:],
                            attnT_sb[ci][:, ts(qt, 128)],
                            wo_sb[:, ci, ts(ncol, 512)],
                            start=(ci == 0),
                            stop=(ci == 1),
                        )
                    nc.vector.tensor_copy(o_sb[:, ts(ncol, 512)], pu[:])
                nc.sync.dma_start(out_ext.ap()[ts(qt, 128), :], o_sb[:])

            # --- schedule ---
            # Engines execute strictly in program order; the exp stream on
            # the scalar engine is ~2x slower than the packed score matmuls
            # feeding it, so independent matmul work (projection chains of
            # the next chunk, the previous chunk's out-projection, P@V of
            # already-exp'd pairs) is threaded between score groups to keep
            # the PE busy while score PSUM bufs wait on exp drains.
            c0 = proj_chunks(0)  # [Q0, K0, Q1, K1, V0..V3]
            pt_q0 = {0: [None] * JPQ, 1: [None] * JPQ}
            c0[0]()
            c0[1]()
            c0[4]()  # V0 covers the K0 psum->SBUF copy latency
            for kt in range(JPQ):
                score_group(0, 0, kt, pt_q0[0])
            c0[2]()
            c0[3]()
            c0[5]()
            for kt in range(JPQ):
                score_group(0, 1, kt, pt_q0[1])
            c0[6]()
            c0[7]()

            # Cross-chunk software pipeline: chunk qc+1's pair-0 score
            # groups (and their exps) are emitted during chunk qc, using the
            # scalar-engine slack of the PE-bound early chunks to pre-drain
            # the exp hump of the (otherwise exp-bound) later chunks.
            pts_cur = pt_q0
            for qc in range(NQC):
                n_kt = (qc + 1) * JPQ
                filler = proj_chunks(qc + 1) if qc + 1 < NQC else []
                fi = 0
                nxt = (
                    {0: [None] * (qc + 2) * JPQ, 1: [None] * (qc + 2) * JPQ}
                    if qc + 1 < NQC
                    else None
                )
                sgn_i = 0

                def fill(n):
                    nonlocal fi
                    for _ in range(n):
                        if fi < len(filler):
                            filler[fi]()
                            fi += 1

                def sgn(n):
                    nonlocal sgn_i
                    if nxt is None:
                        return
                    for _ in range(n):
                        if sgn_i < (qc + 2) * JPQ:
                            score_group(qc + 1, 0, sgn_i, nxt[0])
                            sgn_i += 1

                def oprev(j):
                    if qc > 0:
                        out_proj_tile(qc - 1, j)

                if qc > 0:
                    q4 = max(1, n_kt // 4)
                    for kt in range(n_kt):
                        score_group(qc, 1, kt, pts_cur[1])
                        if kt == q4:
                            fill(1)  # Q0 of qc+1
                        if kt == 2 * q4:
                            oprev(0)
                        if kt == 3 * q4:
                            fill(1)  # K0 of qc+1
                else:
                    fill(2)  # Q0, K0 of chunk 1
                pv_norm(qc, 0, 0, pts_cur[0])
                sgn(3)
                fill(1)  # Q1
                oprev(1)
                pv_norm(qc, 0, 1, pts_cur[0])
                sgn(3)
                fill(1)  # K1
                oprev(2)
                pv_norm(qc, 1, 0, pts_cur[1])
                sgn(3)
                fill(2)  # V0, V1
                pv_norm(qc, 1, 1, pts_cur[1])
                sgn(3)
                fill(len(filler) - fi)
                sgn(99)
                oprev(3)
                if nxt is not None:
                    pts_cur = nxt
            for j in range(JPQ):
                out_proj_tile(NQC - 1, j, pool=s_psum)

    nc.compile()
    return nc


_NC_CACHE = None


def _get_nc():
    global _NC_CACHE
    if _NC_CACHE is None:
        _NC_CACHE = build_nc()
    return _NC_CACHE


def make_in_maps(x, Wq, bq, Wk, bk, Wv, bv, Wo, bo):
    bf = ml_dtypes.bfloat16
    eye = np.eye(128, dtype=bf)
    msk = np.where(
        np.arange(128)[:, None] > np.arange(128)[None, :], MASKVAL, 0.0
    ).astype(bf)
    in_maps = []
    for c in range(N_CORES):
        b, g = c // HPC, c % HPC
        lo, hi = g * DSL, (g + 1) * DSL
        in_maps.append(
            {
                "xT": np.ascontiguousarray(x[b].T).astype(bf),
                "wq": np.ascontiguousarray(Wq[:, lo:hi]).astype(bf),
                "wk": np.ascontiguousarray(Wk[:, lo:hi]).astype(bf),
                "wv": np.ascontiguousarray(Wv[:, lo:hi]).astype(bf),
                "wo": np.ascontiguousarray(Wo[lo:hi, :]).astype(bf),
                "bq": np.ascontiguousarray(bq[lo:hi]).astype(np.float32),
                "eye": eye,
                "msk": msk,
            }
        )
    return in_maps


def gather_output(results, bv, Wo, bo):
    # softmax rows sum to 1, so the V-bias contributes bv @ Wo to every row
    corr = (np.asarray(bv, np.float64) @ np.asarray(Wo, np.float64)).astype(
        np.float32
    ) + np.asarray(bo, np.float32)
    out = np.empty((B, S, D), np.float32)
    for b in range(B):
        acc = np.zeros((S, D), np.float32)
        for g in range(HPC):
            acc += results[b * HPC + g]["out"].astype(np.float32)
        out[b] = acc + corr
    return out


def kernel(x, Wq, bq, Wk, bk, Wv, bv, Wo, bo, _trace=False):
    x = np.asarray(x, np.float32)
    nc = _get_nc()
    in_maps = make_in_maps(x, Wq, bq, Wk, bk, Wv, bv, Wo, bo)
    res = run_bass_kernel_spmd(nc, in_maps, list(range(N_CORES)), trace=_trace)
    out = gather_output(res.results, bv, Wo, bo)
    if _trace:
        return out, res
    return out
